# revision 39
# baseline (speedup 1.0000x reference)
"""Causal single-head attention on 8 Trainium2 NeuronCores.

Problem: x[B=4,T=4096,C=512] fp32, Wk/Wq/Wv[C,H=64] -> out[B,T,H].

Sharding: 2 cores per batch element. Within a pair, the KEY tiles (128 keys
each, 32 tiles) are interleaved by parity: core parity p owns key tiles
{p, p+2, p+4, ...}. Each core computes, for ALL queries of its batch, the
unnormalized partial softmax numerator (sum_k exp(s) * v) and denominator
(sum_k exp(s)) over its own keys only; the host sums the two partials and
divides. exp() without max-subtraction is safe here (scores ~ N(0,1)).

This makes every core's program byte-identical (SPMD requirement): the
causal structure is identical for both parities (query block i needs
exactly 2i+2 local key tiles on either parity), and all parity/batch
differences live in the DMA'd data:
  - xt: x[b].T in fp16, columns permuted to [own-parity key tiles | rest]
  - maskm: multiplicative 0/1 fp16 causal masks for the two diagonal tiles
    of each block (applied AFTER exp, off the S->exp critical path)
  - query columns inside a 512-block are streamed in a fixed parity-dependent
    tile order; the host un-permutes the output columns.

Compute is fp16 (10 mantissa bits, full PE rate). PSUM accumulation fp32.

The middle of the kernel is exp-throughput-bound, so the softmax exp is
split across TWO engines:
  - ACT (scalar) does exact exp via activation LUT
  - DVE (vector) does a Schraudolph-style exp for a subset of off-diagonal
    tiles: int16 = round(184.664*s + 15300.7); those bits ARE the fp16
    exp(s/8) up to a +-3% sawtooth (verified round-to-nearest-even on HW).
    End-to-end harness error with this split: ~2.5e-3 (gate 2e-2).

On-device dataflow per core:
  K^T = [Wk|Wk]^T @ xt_kv  (dup across partition halves for row-packed S)
  Q^T = [Wq|Wq]^T @ xt     (all queries)
  V   = via PE transpose-mode of V^T
  per query block qb (512 q), per local key tile PAIR lp in 0..qb:
     S^T[128k,2x512q] = two half-array MMs, one PSUM tile
     P = exp(S*0.125) on ACT or DVE-Schraudolph -> SBUF fp16
     diagonal pair: P *= maskm (DVE fp16)
     O'[65,512q] += [V|1].T @ P halves  (PSUM accumulate)
  block order 1,2,...,7,0 so the drain tail is the 1-pair block.
"""

import os
import numpy as np

B, T, C, H = 4, 4096, 512, 64
NKT = T // 128          # 32 natural key tiles per batch
NLOC = NKT // 2         # 16 local key tiles per core
QB = T // 512           # 8 query blocks
SCALE = float(H) ** -0.5

# Schraudolph fp16 exp-from-bits constants (balanced-bias variant):
# bits16 = round((2^10/ln2)*x + 15360 - 59.27) with x = s*SCALE
SCHRA_A = float(np.float32(2.0 ** 10 / np.log(2.0)) * np.float32(SCALE))
SCHRA_B = float(np.float32(15360.0 - 59.27))

# number of off-diagonal exp tiles (of 28) run on DVE via Schraudolph
DVE_COUNT = 10

_CACHE = {}
LAST_RESULTS = None


def _dve_tiles():
    """Set of (qb, lp) exp tiles assigned to the DVE Schraudolph path,
    spread evenly through the emission order."""
    blocks = [1, 2, 3, 4, 5, 6, 7, 0]
    picks = {round(j * 28 / DVE_COUNT) for j in range(DVE_COUNT)}
    out = set()
    off_idx = 0
    for qb in blocks:
        for lp in range(qb + 1):
            if lp < qb:
                if off_idx in picks:
                    out.add((qb, lp))
                off_idx += 1
    return out


def _build_program():
    from contextlib import ExitStack
    import concourse.tile as tile
    from concourse import bacc, mybir
    from concourse.masks import make_identity

    F32 = mybir.dt.float32
    F16 = mybir.dt.float16
    I16 = mybir.dt.int16
    EXP = mybir.ActivationFunctionType.Exp
    MULT = mybir.AluOpType.mult
    ADD = mybir.AluOpType.add

    dve_tiles = _dve_tiles()

    nc = bacc.Bacc("TRN2", target_bir_lowering=False, debug=False,
                   num_devices=8)

    xt = nc.dram_tensor("xt", (C, T), F16, kind="ExternalInput").ap()
    wkva = nc.dram_tensor("wkva", (128, 512), F16, kind="ExternalInput").ap()
    wkvb = nc.dram_tensor("wkvb", (128, 512), F16, kind="ExternalInput").ap()
    wqq = nc.dram_tensor("wqq", (128, 512), F16, kind="ExternalInput").ap()
    maskm = nc.dram_tensor("maskm", (128, 1024), F16, kind="ExternalInput").ap()
    opart = nc.dram_tensor("opart", (65, T), F32, kind="ExternalOutput").ap()

    with tile.TileContext(nc) as tc, ExitStack() as ctx:
        persist = ctx.enter_context(tc.tile_pool(name="persist", bufs=1))
        xtp = ctx.enter_context(tc.tile_pool(name="xtp", bufs=4))
        vst_p = ctx.enter_context(tc.tile_pool(name="vst", bufs=2))
        p_pool = ctx.enter_context(tc.tile_pool(name="pp", bufs=6))
        out_p = ctx.enter_context(tc.tile_pool(name="outp", bufs=2))
        # PSUM: 3 S-buffers so the two exp engines (ACT + DVE) overlap -
        # with 2, the S->exp WAR chain serialized them (measured 9% overlap)
        ps_big = ctx.enter_context(tc.tile_pool(name="psb", bufs=3, space="PSUM"))
        ps_po = ctx.enter_context(tc.tile_pool(name="pspo", bufs=1, space="PSUM"))
        ps_sm = ctx.enter_context(tc.tile_pool(name="pss", bufs=1, space="PSUM"))

        # ---- persistent SBUF ----
        wkva_sb = persist.tile([128, 4 * 128], F16)
        wkvb_sb = persist.tile([128, 4 * 128], F16)
        wqq_sb = persist.tile([128, 4 * 128], F16)
        mask_sb = persist.tile([128, 1024], F16)
        kTq_sb = persist.tile([128, NLOC * 128], F16)
        qTq_sb = persist.tile([128, T], F16)
        # [V_l | 1] as 16 separate tiles: the XBAR DMA-transpose needs an
        # offset-free destination, and the ones column rides at col 64
        v_tiles = [persist.tile([128, 65], F16, name=f"v_t{i}")
                   for i in range(NLOC)]
        for vt in v_tiles:
            nc.gpsimd.memset(vt[:, 64:65], 1.0)
        ident = persist.tile([128, 128], F16)
        make_identity(nc, ident[:])

        # ---- input DMA: split across both HWDGE rings (sync + scalar),
        # in consumption order: kv half on scalar ring, oth half on sync.
        # First 1024-col chunk of each half feeds blocks 1-3 + projections
        # tb/pb 0..1; second chunk feeds the rest. ----
        xt_sb = []
        for cc in range(4):
            t = xtp.tile([128, T], F16, tag="xt")
            xt_sb.append(t)
        nc.scalar.dma_start(wkva_sb[:], wkva[:])
        nc.scalar.dma_start(wkvb_sb[:], wkvb[:])
        nc.sync.dma_start(wqq_sb[:], wqq[:])
        nc.sync.dma_start(mask_sb[:], maskm[:])
        # kv half on the scalar ring, oth half on sync, in 512-col chunks
        # ordered to match the projection/attention consumption order
        for lo, hi in ((0, 512), (512, 1024), (1024, 1536), (1536, 2048)):
            for cc in range(4):
                nc.scalar.dma_start(xt_sb[cc][:, lo:hi],
                                    xt[128 * cc:128 * (cc + 1), lo:hi])
            for cc in range(4):
                nc.sync.dma_start(xt_sb[cc][:, 2048 + lo:2048 + hi],
                                  xt[128 * cc:128 * (cc + 1),
                                     2048 + lo:2048 + hi])

        # PE warmup during the DMA preamble: cold back-to-back FULL-ARRAY
        # matmuls (K=128 - half-array ones were observed not to trip the
        # HAM activity monitor) so the clock-gate is released (K=8/8)
        # before real compute starts
        warm_sc = persist.tile([128, 128], F16)
        nc.gpsimd.memset(warm_sc[:], 0.0)
        pwarm = ps_sm.tile([128, 128], F32, tag="sm")
        for _w in range(50):
            nc.tensor.matmul(pwarm[:], warm_sc[:], warm_sc[:],
                             start=True, stop=True, skip_group_check=True)

        qT_v = qTq_sb[:].rearrange("p (half l k) -> p half l k", half=2, k=128)

        kTq_v = kTq_sb[:].rearrange("p (t k) -> p t k", k=128)
        xt_v = [t[:].rearrange("p (t k) -> p t k", k=128) for t in xt_sb]

        def kv_proj(tb):
            # Fused K+V projection in ONE xt stream per C-chunk, with
            # alternating weights: [Wk|Wv] for even local tiles, [Wv|Wk]
            # for odd. K^T thus lands directly on the partition half the
            # row-packed S matmuls read it from (even: 0:64, odd: 64:128)
            # with no duplication pass, and V^T on the opposite half so a
            # single [128,128] PE transpose emits TWO V tiles.
            pkv = ps_sm.tile([128, 512], F32, tag="sm")
            pkv_v = pkv[:].rearrange("p (t k) -> p t k", k=128)
            for cc in range(4):
                nc.tensor.matmul(
                    pkv_v[:, 0:2], wkva_sb[:, 128 * cc:128 * (cc + 1)],
                    xt_v[cc][:, 4 * tb:4 * tb + 3:2, :],
                    start=(cc == 0), stop=(cc == 3))
            for cc in range(4):
                nc.tensor.matmul(
                    pkv_v[:, 2:4], wkvb_sb[:, 128 * cc:128 * (cc + 1)],
                    xt_v[cc][:, 4 * tb + 1:4 * tb + 4:2, :],
                    start=(cc == 0), stop=(cc == 3))
            nc.vector.tensor_copy(kTq_v[0:64, 4 * tb:4 * tb + 3:2, :],
                                  pkv_v[0:64, 0:2])
            nc.vector.tensor_copy(kTq_v[64:128, 4 * tb + 1:4 * tb + 4:2, :],
                                  pkv_v[64:128, 2:4])
            vt2 = vst_p.tile([128, 256], F16, tag="vst")
            nc.vector.tensor_copy(vt2[64:128, :], pkv[64:128, 0:256])
            nc.vector.tensor_copy(vt2[0:64, :], pkv[0:64, 256:512])
            for c in range(2):
                pv = ps_sm.tile([128, 128], F16, tag="sm")
                nc.tensor.transpose(pv[:], vt2[:, 128 * c:128 * (c + 1)],
                                    ident[:])
                nc.vector.tensor_copy(v_tiles[4 * tb + 2 * c][:, 0:64],
                                      pv[:, 64:128])
                nc.vector.tensor_copy(v_tiles[4 * tb + 2 * c + 1][:, 0:64],
                                      pv[:, 0:64])

        def q_proj(pb):
            # [Wq|Wq]: Q^T duplicated across both partition halves
            pqq = ps_big.tile([128, 1024], F32, tag="big")
            for cc in range(4):
                nc.tensor.matmul(
                    pqq[:, 0:512], wqq_sb[:, 128 * cc:128 * (cc + 1)],
                    xt_sb[cc][:, 512 * pb:512 * (pb + 1)],
                    start=(cc == 0), stop=(cc == 3))
            nc.vector.tensor_copy(qTq_sb[:, 512 * pb:512 * (pb + 1)],
                                  pqq[:, 0:512])

        # ---- attention: one flat S->exp->AV pipeline carried across all
        # q-blocks; exp split between ACT (exact) and DVE (Schraudolph) ----
        po_t = {}

        def emit_S(qb, lp):
            q_lo = qT_v[0:64, :, 2 * qb:2 * qb + 2, :]
            q_hi = qT_v[64:128, :, 2 * qb:2 * qb + 2, :]
            ps = ps_big.tile([128, 1024], F32, tag="big")
            l0, l1 = 2 * lp, 2 * lp + 1
            nc.tensor.matmul(ps[:, 0:512],
                             kTq_sb[0:64, 128 * l0:128 * (l0 + 1)],
                             q_lo, start=True, stop=True,
                             tile_position=(0, 0))
            nc.tensor.matmul(ps[:, 512:1024],
                             kTq_sb[64:128, 128 * l1:128 * (l1 + 1)],
                             q_hi, start=True, stop=True,
                             tile_position=(64, 0))
            return ps

        def emit_exp(qb, lp, ps):
            # NOTE: nothing emitted here may WAIT on slow events in the
            # Vector queue - the diagonal mask-mul is deferred to AV time
            # so queued DVE exps behind it aren't blocked (FIFO queues)
            p_sb = p_pool.tile([128, 1024], F16, tag="p")
            if (qb, lp) in dve_tiles:
                nc.vector.tensor_scalar(p_sb[:].bitcast(I16), ps[:],
                                        SCHRA_A, SCHRA_B, MULT, ADD)
            else:
                nc.scalar.activation(p_sb[:], ps[:], EXP, scale=SCALE)
            return p_sb

        def emit_AV(qb, lp, p_sb):
            if lp == qb:  # diagonal: 0/1 mask (fp16 2x) right before use
                nc.vector.tensor_tensor(p_sb[:], p_sb[:], mask_sb[:], MULT)
            if lp == 0:
                po_new = ps_po.tile([65, 512], F32, tag="po")
                po_t[qb] = po_new
            po = po_t[qb]
            for h in range(2):
                l = 2 * lp + h
                nc.tensor.matmul(po[:], v_tiles[l][:],
                                 p_sb[:, 512 * h:512 * (h + 1)],
                                 start=(l == 0), stop=(l == 2 * qb + 1))

        from collections import deque
        s_q = deque()   # (qb, lp, ps) awaiting exp
        e_q = deque()   # (qb, lp, p_sb) awaiting AV
        out_q = deque()  # (qb, po) awaiting out copy+DMA

        def emit_out(qb, po):
            # emitted a few pump steps after the block's last AV so the
            # copy's semaphore wait is already satisfied when it reaches
            # the head of the Vector queue
            o_sb = out_p.tile([65, 512], F32, tag="o")
            nc.vector.tensor_copy(o_sb[:], po[:])
            nc.sync.dma_start(opart[:, 512 * qb:512 * (qb + 1)], o_sb[:])

        def pump():
            if len(s_q) >= 2:
                qb, lp, ps = s_q.popleft()
                e_q.append((qb, lp, emit_exp(qb, lp, ps)))
            if len(e_q) >= 2:
                if out_q:  # must precede the next po allocation (bufs=1)
                    emit_out(*out_q.popleft())
                qb, lp, p_sb = e_q.popleft()
                emit_AV(qb, lp, p_sb)
                if lp == qb:
                    out_q.append((qb, po_t.pop(qb)))

        def attn_block(qb):
            for lp in range(qb + 1):
                s_q.append((qb, lp, emit_S(qb, lp)))
                pump()

        def flush_exps():
            # drain pending exps so a following projection group's PSUM
            # CASTs (which wait on proj matmuls) don't block them in the
            # Vector/Scalar queues
            while s_q:
                qb, lp, ps = s_q.popleft()
                e_q.append((qb, lp, emit_exp(qb, lp, ps)))
            while len(e_q) > 2:
                if out_q:
                    emit_out(*out_q.popleft())
                qb, lp, p_sb = e_q.popleft()
                emit_AV(qb, lp, p_sb)
                if lp == qb:
                    out_q.append((qb, po_t.pop(qb)))

        # projections woven between attention blocks as PE filler, always at
        # least one block ahead of their consumers; block order 1..7 then 0
        # so the drain tail is the 1-pair block
        kv_proj(0); q_proj(0); q_proj(4)
        attn_block(1)
        flush_exps()
        kv_proj(1); q_proj(1); q_proj(5)
        attn_block(2)
        attn_block(3)
        flush_exps()
        kv_proj(2); q_proj(2); q_proj(6)
        attn_block(4)
        attn_block(5)
        flush_exps()
        kv_proj(3); q_proj(3); q_proj(7)
        attn_block(6)
        attn_block(7)
        attn_block(0)
        while s_q or e_q or out_q:
            if out_q:
                emit_out(*out_q.popleft())
            if s_q:
                qb, lp, ps = s_q.popleft()
                e_q.append((qb, lp, emit_exp(qb, lp, ps)))
            if e_q:
                qb, lp, p_sb = e_q.popleft()
                emit_AV(qb, lp, p_sb)
                if lp == qb:
                    out_q.append((qb, po_t.pop(qb)))

    nc.compile()
    return nc


def _prep_inputs(x, Wk, Wq, Wv):
    """Per-core input marshalling (layout + fp16 cast, no math)."""
    def swz(w):
        # [C, m] -> [128, 4*m]: chunk cc (rows 128cc..) at free cols m*cc..
        m = w.shape[1]
        return np.ascontiguousarray(
            w.reshape(4, 128, m).transpose(1, 0, 2).reshape(128, 4 * m)
        ).astype(np.float16)

    wkva = swz(np.concatenate([Wk, Wv], axis=1))
    wkvb = swz(np.concatenate([Wv, Wk], axis=1))
    wqq = swz(np.concatenate([Wq, Wq], axis=1))
    mask_cache = {}
    in_maps = []
    for core in range(8):
        b, par = core // 2, core % 2
        xT = np.ascontiguousarray(x[b].T).astype(np.float16)   # [C, T]
        tiles = xT.reshape(C, NKT, 128)
        kv = tiles[:, par::2, :].reshape(C, NLOC * 128)
        oth = tiles[:, 1 - par::2, :].reshape(C, NLOC * 128)
        xt_perm = np.ascontiguousarray(np.concatenate([kv, oth], axis=1))

        if par not in mask_cache:
            J = [par, 2 + par, 1 - par, 3 - par]
            m = np.zeros((128, 1024), np.float16)
            ks = np.arange(128)[:, None]
            qr = np.arange(128)[None, :]
            for mi, off in enumerate((par, 2 + par)):
                for s in range(4):
                    cond = (128 * off + ks) <= (128 * J[s] + qr)
                    m[:, 512 * mi + 128 * s:512 * mi + 128 * (s + 1)] = \
                        np.where(cond, np.float16(1.0), np.float16(0.0))
            mask_cache[par] = m
        in_maps.append({"xt": xt_perm, "wkva": wkva, "wkvb": wkvb,
                        "wqq": wqq, "maskm": mask_cache[par]})
    return in_maps


def _combine(results):
    """Un-permute query columns, sum partials across the core pairs, divide."""
    out = np.empty((B, T, H), np.float32)
    for b in range(4):
        nats = []
        for par in range(2):
            J = [par, 2 + par, 1 - par, 3 - par]
            r = results[2 * b + par]["opart"].reshape(65, QB, 4, 128)
            nat = np.empty_like(r)
            for s in range(4):
                nat[:, :, J[s], :] = r[:, :, s, :]
            nats.append(nat.reshape(65, T))
        num = nats[0][:64] + nats[1][:64]
        den = nats[0][64] + nats[1][64]
        out[b] = (num / den[None, :]).T
    return out


def kernel(x, Wk, Wq, Wv):
    global LAST_RESULTS
    from concourse.bass_utils import run_bass_kernel_spmd

    if "nc" not in _CACHE:
        _CACHE["nc"] = _build_program()
    nc = _CACHE["nc"]

    in_maps = _prep_inputs(np.asarray(x, np.float32), np.asarray(Wk),
                           np.asarray(Wq), np.asarray(Wv))
    trace = bool(int(os.environ.get("ATTN_TRACE", "0")))
    res = run_bass_kernel_spmd(nc, in_maps, core_ids=list(range(8)),
                               trace=trace)
    LAST_RESULTS = res
    return _combine(res.results)


if __name__ == "__main__":
    rng = np.random.default_rng(0)
    x = rng.standard_normal((B, T, C), dtype=np.float32)
    Wk = (rng.standard_normal((C, H)) * C ** -0.5).astype(np.float32)
    Wq = (rng.standard_normal((C, H)) * C ** -0.5).astype(np.float32)
    Wv = (rng.standard_normal((C, H)) * C ** -0.5).astype(np.float32)
    out = kernel(x, Wk, Wq, Wv)
    k = x @ Wk; q = x @ Wq; v = x @ Wv
    s = np.einsum('bqh,bkh->bqk', q, k) * SCALE
    mask = np.tril(np.ones((T, T), dtype=bool))
    s = np.where(mask, s, -np.inf)
    p = np.exp(s - s.max(-1, keepdims=True))
    p /= p.sum(-1, keepdims=True)
    ref = np.einsum('bqk,bkh->bqh', p, v)
    err = np.abs(out - ref).max() / np.abs(ref).max()
    print("rel err vs numpy:", err)


# revision 43
# speedup vs baseline: 1.0943x; 1.0943x over previous
"""Causal single-head attention on 8 Trainium2 NeuronCores.

Problem: x[B=4,T=4096,C=512] fp32, Wk/Wq/Wv[C,H=64] -> out[B,T,H].

Sharding: 2 cores per batch element. Within a pair, the KEY tiles (128 keys
each, 32 tiles) are interleaved by parity: core parity p owns key tiles
{p, p+2, p+4, ...}. Each core computes, for ALL queries of its batch, the
unnormalized partial softmax numerator (sum_k exp(s) * v) and denominator
(sum_k exp(s)) over its own keys only; the host sums the two partials and
divides. exp() without max-subtraction is safe here (scores ~ N(0,1)).

This makes every core's program byte-identical (SPMD requirement): the
causal structure is identical for both parities (query block i needs
exactly 2i+2 local key tiles on either parity), and all parity/batch
differences live in the DMA'd data:
  - xt: x[b].T in fp16, columns permuted to [own-parity key tiles | rest]
  - maskm: multiplicative 0/1 fp16 causal masks for the two diagonal tiles
    of each block (applied AFTER exp, off the S->exp critical path)
  - query columns inside a 512-block are streamed in a fixed parity-dependent
    tile order; the host un-permutes the output columns.

Compute is fp16 (10 mantissa bits, full PE rate). PSUM accumulation fp32.

The middle of the kernel is exp-throughput-bound, so the softmax exp is
split across TWO engines:
  - ACT (scalar) does exact exp via activation LUT
  - DVE (vector) does a Schraudolph-style exp for a subset of off-diagonal
    tiles: int16 = round(184.664*s + 15300.7); those bits ARE the fp16
    exp(s/8) up to a +-3% sawtooth (verified round-to-nearest-even on HW).
    End-to-end harness error with this split: ~2.5e-3 (gate 2e-2).

On-device dataflow per core:
  K^T = [Wk|Wk]^T @ xt_kv  (dup across partition halves for row-packed S)
  Q^T = [Wq|Wq]^T @ xt     (all queries)
  V   = via PE transpose-mode of V^T
  per query block qb (512 q), per local key tile PAIR lp in 0..qb:
     S^T[128k,2x512q] = two half-array MMs, one PSUM tile
     P = exp(S*0.125) on ACT or DVE-Schraudolph -> SBUF fp16
     diagonal pair: P *= maskm (DVE fp16)
     O'[65,512q] += [V|1].T @ P halves  (PSUM accumulate)
  block order 1,2,...,7,0 so the drain tail is the 1-pair block.
"""

import os
import numpy as np

B, T, C, H = 4, 4096, 512, 64
NKT = T // 128          # 32 natural key tiles per batch
NLOC = NKT // 2         # 16 local key tiles per core
QB = T // 512           # 8 query blocks
SCALE = float(H) ** -0.5

# Schraudolph fp16 exp-from-bits constants (balanced-bias variant):
# bits16 = round((2^10/ln2)*x + 15360 - 59.27) with x = s*SCALE
SCHRA_A = float(np.float32(2.0 ** 10 / np.log(2.0)) * np.float32(SCALE))
SCHRA_B = float(np.float32(15360.0 - 59.27))

# number of off-diagonal exp tiles (of 28) run on DVE via Schraudolph
DVE_COUNT = 10

_CACHE = {}
LAST_RESULTS = None


def _dve_tiles():
    """Set of (qb, lp) exp tiles assigned to the DVE Schraudolph path,
    spread evenly through the emission order."""
    blocks = [1, 2, 3, 4, 5, 6, 7, 0]
    picks = {round(j * 28 / DVE_COUNT) for j in range(DVE_COUNT)}
    out = set()
    off_idx = 0
    for qb in blocks:
        for lp in range(qb + 1):
            if lp < qb:
                if off_idx in picks:
                    out.add((qb, lp))
                off_idx += 1
    return out


def _build_program():
    from contextlib import ExitStack
    import concourse.tile as tile
    from concourse import bacc, mybir
    from concourse.masks import make_identity

    F32 = mybir.dt.float32
    F16 = mybir.dt.float16
    I16 = mybir.dt.int16
    EXP = mybir.ActivationFunctionType.Exp
    MULT = mybir.AluOpType.mult
    ADD = mybir.AluOpType.add

    dve_tiles = _dve_tiles()

    nc = bacc.Bacc("TRN2", target_bir_lowering=False, debug=False,
                   num_devices=8)

    xt = nc.dram_tensor("xt", (C, T), F16, kind="ExternalInput").ap()
    wkva = nc.dram_tensor("wkva", (128, 512), F16, kind="ExternalInput").ap()
    wkvb = nc.dram_tensor("wkvb", (128, 512), F16, kind="ExternalInput").ap()
    wqq = nc.dram_tensor("wqq", (128, 512), F16, kind="ExternalInput").ap()
    maskm = nc.dram_tensor("maskm", (128, 1024), F16, kind="ExternalInput").ap()
    opart = nc.dram_tensor("opart", (65, T), F32, kind="ExternalOutput").ap()

    with tile.TileContext(nc) as tc, ExitStack() as ctx:
        persist = ctx.enter_context(tc.tile_pool(name="persist", bufs=1))
        xtp = ctx.enter_context(tc.tile_pool(name="xtp", bufs=4))
        vst_p = ctx.enter_context(tc.tile_pool(name="vst", bufs=2))
        p_pool = ctx.enter_context(tc.tile_pool(name="pp", bufs=6))
        out_p = ctx.enter_context(tc.tile_pool(name="outp", bufs=2))
        # PSUM: 3 S-buffers so the two exp engines (ACT + DVE) overlap -
        # with 2, the S->exp WAR chain serialized them (measured 9% overlap)
        ps_big = ctx.enter_context(tc.tile_pool(name="psb", bufs=3, space="PSUM"))
        ps_po = ctx.enter_context(tc.tile_pool(name="pspo", bufs=1, space="PSUM"))
        ps_sm = ctx.enter_context(tc.tile_pool(name="pss", bufs=1, space="PSUM"))

        # ---- persistent SBUF ----
        wkva_sb = persist.tile([128, 4 * 128], F16)
        wkvb_sb = persist.tile([128, 4 * 128], F16)
        wqq_sb = persist.tile([128, 4 * 128], F16)
        mask_sb = persist.tile([128, 1024], F16)
        kTq_sb = persist.tile([128, NLOC * 128], F16)
        qTq_sb = persist.tile([128, T], F16)
        # [V_l | 1] as 16 separate tiles: the XBAR DMA-transpose needs an
        # offset-free destination, and the ones column rides at col 64
        v_tiles = [persist.tile([128, 65], F16, name=f"v_t{i}")
                   for i in range(NLOC)]
        for vt in v_tiles:
            nc.gpsimd.memset(vt[:, 64:65], 1.0)
        ident = persist.tile([128, 128], F16)
        make_identity(nc, ident[:])

        # ---- input DMA: split across both HWDGE rings (sync + scalar),
        # in consumption order: kv half on scalar ring, oth half on sync.
        # First 1024-col chunk of each half feeds blocks 1-3 + projections
        # tb/pb 0..1; second chunk feeds the rest. ----
        xt_sb = []
        for cc in range(4):
            t = xtp.tile([128, T], F16, tag="xt")
            xt_sb.append(t)
        # weights + mask go via the (idle) GpSimd SWDGE ring so both HW
        # rings start streaming xt immediately
        nc.gpsimd.dma_start(wkva_sb[:], wkva[:])
        nc.gpsimd.dma_start(wkvb_sb[:], wkvb[:])
        nc.gpsimd.dma_start(wqq_sb[:], wqq[:])
        nc.gpsimd.dma_start(mask_sb[:], maskm[:])
        # kv half on the scalar ring, oth half on sync, in 512-col chunks
        # ordered to match the projection/attention consumption order
        for lo, hi in ((0, 512), (512, 1024), (1024, 1536), (1536, 2048)):
            for cc in range(4):
                nc.scalar.dma_start(xt_sb[cc][:, lo:hi],
                                    xt[128 * cc:128 * (cc + 1), lo:hi])
            for cc in range(4):
                nc.sync.dma_start(xt_sb[cc][:, 2048 + lo:2048 + hi],
                                  xt[128 * cc:128 * (cc + 1),
                                     2048 + lo:2048 + hi])

        # PE warmup during the DMA preamble: cold back-to-back FULL-ARRAY
        # matmuls (K=128 - half-array ones were observed not to trip the
        # HAM activity monitor) so the clock-gate is released (K=8/8)
        # before real compute starts
        warm_sc = persist.tile([128, 128], F16)
        nc.gpsimd.memset(warm_sc[:], 0.0)
        pwarm = ps_sm.tile([128, 128], F32, tag="sm")
        for _w in range(36):
            nc.tensor.matmul(pwarm[:], warm_sc[:], warm_sc[:],
                             start=True, stop=True, skip_group_check=True)

        qT_v = qTq_sb[:].rearrange("p (half l k) -> p half l k", half=2, k=128)

        kTq_v = kTq_sb[:].rearrange("p (t k) -> p t k", k=128)
        xt_v = [t[:].rearrange("p (t k) -> p t k", k=128) for t in xt_sb]

        def kv_proj(tb):
            # Fused K+V projection in ONE xt stream per C-chunk, with
            # alternating weights: [Wk|Wv] for even local tiles, [Wv|Wk]
            # for odd. K^T thus lands directly on the partition half the
            # row-packed S matmuls read it from (even: 0:64, odd: 64:128)
            # with no duplication pass, and V^T on the opposite half so a
            # single [128,128] PE transpose emits TWO V tiles.
            pkv = ps_sm.tile([128, 512], F32, tag="sm")
            pkv_v = pkv[:].rearrange("p (t k) -> p t k", k=128)
            for cc in range(4):
                nc.tensor.matmul(
                    pkv_v[:, 0:2], wkva_sb[:, 128 * cc:128 * (cc + 1)],
                    xt_v[cc][:, 4 * tb:4 * tb + 3:2, :],
                    start=(cc == 0), stop=(cc == 3))
            for cc in range(4):
                nc.tensor.matmul(
                    pkv_v[:, 2:4], wkvb_sb[:, 128 * cc:128 * (cc + 1)],
                    xt_v[cc][:, 4 * tb + 1:4 * tb + 4:2, :],
                    start=(cc == 0), stop=(cc == 3))
            nc.vector.tensor_copy(kTq_v[0:64, 4 * tb:4 * tb + 3:2, :],
                                  pkv_v[0:64, 0:2])
            nc.vector.tensor_copy(kTq_v[64:128, 4 * tb + 1:4 * tb + 4:2, :],
                                  pkv_v[64:128, 2:4])
            vt2 = vst_p.tile([128, 256], F16, tag="vst")
            nc.vector.tensor_copy(vt2[64:128, :], pkv[64:128, 0:256])
            nc.vector.tensor_copy(vt2[0:64, :], pkv[0:64, 256:512])
            for c in range(2):
                pv = ps_sm.tile([128, 128], F16, tag="sm")
                nc.tensor.transpose(pv[:], vt2[:, 128 * c:128 * (c + 1)],
                                    ident[:])
                nc.vector.tensor_copy(v_tiles[4 * tb + 2 * c][:, 0:64],
                                      pv[:, 64:128])
                nc.vector.tensor_copy(v_tiles[4 * tb + 2 * c + 1][:, 0:64],
                                      pv[:, 0:64])

        def q_proj(pb):
            # [Wq|Wq]: Q^T duplicated across both partition halves
            pqq = ps_big.tile([128, 1024], F32, tag="big")
            for cc in range(4):
                nc.tensor.matmul(
                    pqq[:, 0:512], wqq_sb[:, 128 * cc:128 * (cc + 1)],
                    xt_sb[cc][:, 512 * pb:512 * (pb + 1)],
                    start=(cc == 0), stop=(cc == 3))
            nc.vector.tensor_copy(qTq_sb[:, 512 * pb:512 * (pb + 1)],
                                  pqq[:, 0:512])

        # ---- attention: one flat S->exp->AV pipeline carried across all
        # q-blocks; exp split between ACT (exact) and DVE (Schraudolph) ----
        po_t = {}

        def emit_S(qb, lp):
            q_lo = qT_v[0:64, :, 2 * qb:2 * qb + 2, :]
            q_hi = qT_v[64:128, :, 2 * qb:2 * qb + 2, :]
            ps = ps_big.tile([128, 1024], F32, tag="big")
            l0, l1 = 2 * lp, 2 * lp + 1
            nc.tensor.matmul(ps[:, 0:512],
                             kTq_sb[0:64, 128 * l0:128 * (l0 + 1)],
                             q_lo, start=True, stop=True,
                             tile_position=(0, 0))
            nc.tensor.matmul(ps[:, 512:1024],
                             kTq_sb[64:128, 128 * l1:128 * (l1 + 1)],
                             q_hi, start=True, stop=True,
                             tile_position=(64, 0))
            return ps

        def emit_exp(qb, lp, ps):
            p_sb = p_pool.tile([128, 1024], F16, tag="p")
            if (qb, lp) in dve_tiles:
                nc.vector.tensor_scalar(p_sb[:].bitcast(I16), ps[:],
                                        SCHRA_A, SCHRA_B, MULT, ADD)
            else:
                nc.scalar.activation(p_sb[:], ps[:], EXP, scale=SCALE)
                if lp == qb:  # diagonal pair: 0/1 mask AFTER exp, fp16 2x
                    nc.vector.tensor_tensor(p_sb[:], p_sb[:], mask_sb[:],
                                            MULT)
            return p_sb

        def emit_AV(qb, lp, p_sb):
            if lp == 0:
                po_new = ps_po.tile([65, 512], F32, tag="po")
                po_t[qb] = po_new
            po = po_t[qb]
            for h in range(2):
                l = 2 * lp + h
                nc.tensor.matmul(po[:], v_tiles[l][:],
                                 p_sb[:, 512 * h:512 * (h + 1)],
                                 start=(l == 0), stop=(l == 2 * qb + 1))
            if lp == qb:  # block done
                po = po_t.pop(qb)
                o_sb = out_p.tile([65, 512], F32, tag="o")
                nc.vector.tensor_copy(o_sb[:], po[:])
                nc.sync.dma_start(opart[:, 512 * qb:512 * (qb + 1)], o_sb[:])

        from collections import deque
        s_q = deque()   # (qb, lp, ps) awaiting exp
        e_q = deque()   # (qb, lp, p_sb) awaiting AV

        def pump():
            if len(s_q) >= 2:
                qb, lp, ps = s_q.popleft()
                e_q.append((qb, lp, emit_exp(qb, lp, ps)))
            if len(e_q) >= 2:
                qb, lp, p_sb = e_q.popleft()
                emit_AV(qb, lp, p_sb)

        def attn_block(qb):
            for lp in range(qb + 1):
                s_q.append((qb, lp, emit_S(qb, lp)))
                pump()

        # projections woven between attention blocks as PE filler, always at
        # least one block ahead of their consumers; block order 1..7 then 0
        # so the drain tail is the 1-pair block
        kv_proj(0); q_proj(0); q_proj(4)
        attn_block(1)
        kv_proj(1); q_proj(1); q_proj(5)
        attn_block(2)
        attn_block(3)
        kv_proj(2); q_proj(2); q_proj(6)
        attn_block(4)
        attn_block(5)
        kv_proj(3); q_proj(3); q_proj(7)
        attn_block(6)
        attn_block(7)
        attn_block(0)
        while s_q or e_q:
            if s_q:
                qb, lp, ps = s_q.popleft()
                e_q.append((qb, lp, emit_exp(qb, lp, ps)))
            if e_q:
                qb, lp, p_sb = e_q.popleft()
                emit_AV(qb, lp, p_sb)

    nc.compile()
    return nc


def _prep_inputs(x, Wk, Wq, Wv):
    """Per-core input marshalling (layout + fp16 cast, no math)."""
    def swz(w):
        # [C, m] -> [128, 4*m]: chunk cc (rows 128cc..) at free cols m*cc..
        m = w.shape[1]
        return np.ascontiguousarray(
            w.reshape(4, 128, m).transpose(1, 0, 2).reshape(128, 4 * m)
        ).astype(np.float16)

    wkva = swz(np.concatenate([Wk, Wv], axis=1))
    wkvb = swz(np.concatenate([Wv, Wk], axis=1))
    wqq = swz(np.concatenate([Wq, Wq], axis=1))
    mask_cache = {}
    in_maps = []
    for core in range(8):
        b, par = core // 2, core % 2
        xT = np.ascontiguousarray(x[b].T).astype(np.float16)   # [C, T]
        tiles = xT.reshape(C, NKT, 128)
        kv = tiles[:, par::2, :].reshape(C, NLOC * 128)
        oth = tiles[:, 1 - par::2, :].reshape(C, NLOC * 128)
        xt_perm = np.ascontiguousarray(np.concatenate([kv, oth], axis=1))

        if par not in mask_cache:
            J = [par, 2 + par, 1 - par, 3 - par]
            m = np.zeros((128, 1024), np.float16)
            ks = np.arange(128)[:, None]
            qr = np.arange(128)[None, :]
            for mi, off in enumerate((par, 2 + par)):
                for s in range(4):
                    cond = (128 * off + ks) <= (128 * J[s] + qr)
                    m[:, 512 * mi + 128 * s:512 * mi + 128 * (s + 1)] = \
                        np.where(cond, np.float16(1.0), np.float16(0.0))
            mask_cache[par] = m
        in_maps.append({"xt": xt_perm, "wkva": wkva, "wkvb": wkvb,
                        "wqq": wqq, "maskm": mask_cache[par]})
    return in_maps


def _combine(results):
    """Un-permute query columns, sum partials across the core pairs, divide."""
    out = np.empty((B, T, H), np.float32)
    for b in range(4):
        nats = []
        for par in range(2):
            J = [par, 2 + par, 1 - par, 3 - par]
            r = results[2 * b + par]["opart"].reshape(65, QB, 4, 128)
            nat = np.empty_like(r)
            for s in range(4):
                nat[:, :, J[s], :] = r[:, :, s, :]
            nats.append(nat.reshape(65, T))
        num = nats[0][:64] + nats[1][:64]
        den = nats[0][64] + nats[1][64]
        out[b] = (num / den[None, :]).T
    return out


def kernel(x, Wk, Wq, Wv):
    global LAST_RESULTS
    from concourse.bass_utils import run_bass_kernel_spmd

    if "nc" not in _CACHE:
        _CACHE["nc"] = _build_program()
    nc = _CACHE["nc"]

    in_maps = _prep_inputs(np.asarray(x, np.float32), np.asarray(Wk),
                           np.asarray(Wq), np.asarray(Wv))
    trace = bool(int(os.environ.get("ATTN_TRACE", "0")))
    res = run_bass_kernel_spmd(nc, in_maps, core_ids=list(range(8)),
                               trace=trace)
    LAST_RESULTS = res
    return _combine(res.results)


if __name__ == "__main__":
    rng = np.random.default_rng(0)
    x = rng.standard_normal((B, T, C), dtype=np.float32)
    Wk = (rng.standard_normal((C, H)) * C ** -0.5).astype(np.float32)
    Wq = (rng.standard_normal((C, H)) * C ** -0.5).astype(np.float32)
    Wv = (rng.standard_normal((C, H)) * C ** -0.5).astype(np.float32)
    out = kernel(x, Wk, Wq, Wv)
    k = x @ Wk; q = x @ Wq; v = x @ Wv
    s = np.einsum('bqh,bkh->bqk', q, k) * SCALE
    mask = np.tril(np.ones((T, T), dtype=bool))
    s = np.where(mask, s, -np.inf)
    p = np.exp(s - s.max(-1, keepdims=True))
    p /= p.sum(-1, keepdims=True)
    ref = np.einsum('bqk,bkh->bqh', p, v)
    err = np.abs(out - ref).max() / np.abs(ref).max()
    print("rel err vs numpy:", err)
